# revision 17
# baseline (speedup 1.0000x reference)
"""Trainium2 Bass kernel for nn_ApproxCompressor (v15).

Reference (per sample n):
    alpha = sigmoid(z_alpha); h[k] = (1-alpha)*alpha^k (k<16384)
    env   = causal_conv(mean_c x^2, h); LG = log(env + 1e-5)
    quadratic-knee gain; out = gain * x.

Kernel strategy (8 cores x 4 samples, pure data parallel):
  * Time-on-partition layout: sample time t = 128*j + p (partition p,
    column j; 1024 columns per sample).  The exponential FIR truncated at
    256 taps (a^256 < 1e-7, asserted on host) is EXACTLY a 2-tap block
    convolution over columns:
        Y[:, j] = W0 @ D[:, j] + W1 @ D[:, j-1]
    with W0[q,p] = a^(p-q) (p>=q), W1[q,p] = a^(128+p-q), run as bf16
    matmuls on the PE accumulating in PSUM (per-channel squares matmul'd
    separately so no channel pre-add is needed).
  * Relu-free knee: host folds b1 = W - T into the Ln activation
    (scale' = 0.5*(1-alpha)*e^b1, bias' = eps*e^b1) so ACT's Ln emits
    A' = LG - T + W directly from PSUM.  A SINGLE custom DVE op
    (registered at import into dve_ops.OPS) then computes
        Q = c*(2A' - c),  c = min(relu(A'), 2W)
    which equals relu(u+W)^2 - relu(u-W)^2 exactly (c=0 kills the A'<0
    region), entirely in the DVE's fp32 pipeline.  log_gain = c/4W * Q
    is folded into Exp's input scale.  Per sample the DVE runs only
    square (TT 2x), knee (1 custom op), out-mult (TT 2x) ~= 3.7us;
    ACT runs Ln + Exp ~= 2.4us.
  * The Tile scheduler is steered with tile_wait_until step numbers
    (pure sim-time priorities) -- v6 showed it otherwise reorders the
    per-engine queues around its DMA model and stalls the pipeline.
  * DMA (3 rings, ~770KB each input side): sync x0c0,x1c0,x2c0;
    scalar wm0,prm,wmr,x3c0,x3c1 (weights lead so the first matmul is
    never weight-blocked); pool x0c1,x1c1,x2c1.  Stores issue per
    sample as outputs complete (sync/pool), s3's final half on scalar.
  * s0 and s3 are processed in 512-column (PSUM-bank) halves through
    Ln/knee (s3 also Exp/out) to shorten pipeline fill and drain.
"""

import os
import sys

import numpy as np


def _import_concourse():
    try:
        import concourse.bass  # noqa: F401
    except ImportError:
        for p in ("/opt/trn_rl_repo", "/root/.axon_site/_ro/trn_rl_repo"):
            if os.path.isdir(p) and p not in sys.path:
                sys.path.insert(0, p)
        import concourse.bass  # noqa: F401


_import_concourse()

import ml_dtypes  # noqa: E402
import concourse.bass as bass  # noqa: E402
import concourse.tile as tile  # noqa: E402
from concourse import bacc, mybir  # noqa: E402
from concourse import dve_ops as _dvo  # noqa: E402
from concourse.dve_spec import (  # noqa: E402
    C0 as _C0,
    Spec as _Spec,
    Src0 as _Src0,
    lower as _dve_lower,
    minn as _minn,
    relu as _relu,
)
from concourse.dve_uop import DveOpSpec as _DveOpSpec  # noqa: E402

N, C, L = 32, 2, 131072
NCORES = 8
NLOC = N // NCORES  # 4 samples/core
P = 128
COLS = L // P  # 1024 columns per sample
SROW = C * COLS  # 2048 elems per sample per partition row
ROW = NLOC * SROW  # 8192 elems per partition row
EPS = 1e-5
K_FIR = 16384
JF = 256  # FIR truncation (2 blocks of 128)
HB = COLS // 2  # psum bank = 512 f32 columns

F32 = mybir.dt.float32
BF16 = mybir.dt.bfloat16

# per-sample param column slots (prm tile is [P, NLOC*NPRM] f32)
PRM_LNSCALE, PRM_LNBIAS, PRM_W2, PRM_C4W = 0, 1, 2, 3
NPRM = 4
ACT_SET_ID = 6  # natural_log_exp_and_others: ln, exp

TRACE_RESULT = {}


def _knee_ref(in0, in1, s0, s1, imm2):
    x = in0.astype(np.float32)
    c = np.clip(x, 0.0, s0)
    return c * (2.0 * x - c)


def _register_knee_op():
    """Append the fused knee op  Q = c*(2A'-c), c = clip(A', 0, s0)  to
    dve_ops.OPS (the documented extension point for custom DVE ucode).
    Idempotent; shas computed from lower() so they always match."""
    name = "ACOMP_KNEE_V1"
    for op in _dvo.OPS:
        if op.name == name:
            return op
    c = _minn(_relu(_Src0), _C0)
    spec = _Spec(body=c * (_Src0 + _Src0 - c), reference=_knee_ref)
    row = _dvo._CUSTOM_DVE_ROW_BASE + len(_dvo.OPS)
    assert row < 0x20, "custom DVE opcode rows exhausted"
    _dvo._SUB_OPCODE_FOR_NAME[name] = row
    shas = {}
    for ver in ("v3", "v4"):
        s = _DveOpSpec(name=name, opcode=row,
                       uops=_dve_lower(spec, ver=ver), rd1_en=False)
        shas[ver] = s.sha(ver)
    op = _dvo.DveOp(name, spec, subdim=False, uops_sha=shas)
    _dvo.OPS.append(op)
    _dvo.CUSTOM_DVE_SPECS[name] = spec
    return op


KNEE_OP = _register_knee_op()


def build_nc():
    AF = mybir.ActivationFunctionType
    OP = mybir.AluOpType

    nc = bacc.Bacc("TRN2", target_bir_lowering=False, num_devices=NCORES)
    xd_ext = nc.declare_dram_parameter("xd", [P, ROW], BF16, isOutput=False)
    prm_ext = nc.declare_dram_parameter("prm", [P, NLOC * NPRM], F32, isOutput=False)
    wm_ext = nc.declare_dram_parameter("wm", [P, NLOC * 2 * P], BF16, isOutput=False)
    od_ext = nc.declare_dram_parameter("od", [P, ROW], BF16, isOutput=True)

    with tile.TileContext(nc) as tc:
        atl = mybir.InstLoadActFuncSet(
            name=nc.get_next_instruction_name(), ins=[], outs=[],
            act_func_set_id=ACT_SET_ID,
        )
        nc.scalar.add_instruction(atl)
        step = tc.tile_wait_until
        with (
            tc.tile_pool(name="pc", bufs=1) as pc,
            tc.tile_pool(name="pin", bufs=1) as pin,
            tc.tile_pool(name="pk", bufs=1) as pk,
            tc.tile_pool(name="pps", bufs=1, space=bass.MemorySpace.PSUM) as pps,
        ):
            wm0 = pc.tile([P, 2 * P], BF16, tag="wm0")
            wmr = pc.tile([P, 3 * 2 * P], BF16, tag="wmr")
            prm = pc.tile([P, NLOC * NPRM], F32, tag="prm")

            def col(s, k):
                return prm[:, s * NPRM + k : s * NPRM + k + 1]

            xt = [pin.tile([P, SROW], BF16, tag=f"x{s}", name=f"x{s}")
                  for s in range(NLOC)]

            def ldch(s, ch, eng):
                c0 = s * SROW + ch * COLS
                eng.dma_start(out=xt[s][:, ch * COLS : (ch + 1) * COLS],
                              in_=xd_ext[:, c0 : c0 + COLS])

            # ---- input DMA (ring order fixed by step numbers) ------------
            with step(0.01):
                ldch(0, 0, nc.sync)
                nc.scalar.dma_start(out=wm0[:], in_=wm_ext[:, 0 : 2 * P])
                ldch(0, 1, nc.gpsimd)
            with step(0.02):
                nc.scalar.dma_start(out=prm[:], in_=prm_ext[:])
            with step(0.03):
                ldch(1, 0, nc.sync)
                ldch(1, 1, nc.gpsimd)
                nc.scalar.dma_start(out=wmr[:], in_=wm_ext[:, 2 * P :])
            with step(0.05):
                ldch(2, 0, nc.sync)
                ldch(2, 1, nc.gpsimd)
            with step(0.06):
                ldch(3, 0, nc.scalar)
            with step(0.07):
                ldch(3, 1, nc.scalar)

            # ---- tiles ---------------------------------------------------
            sq0 = [pk.tile([P, COLS], BF16, tag=f"sq0c{c_}", name=f"sq0c{c_}")
                   for c_ in range(2)]
            sqs = {s: pk.tile([P, SROW], BF16, tag=f"sq{s}", name=f"sq{s}")
                   for s in range(1, NLOC)}
            aps = {s: pk.tile([P, COLS], BF16, tag=f"ap{s}", name=f"ap{s}")
                   for s in range(NLOC)}
            Qs = {s: pk.tile([P, COLS], BF16, tag=f"Q{s}", name=f"Q{s}")
                  for s in range(NLOC)}
            gs = {s: pk.tile([P, COLS], BF16, tag=f"g{s}", name=f"g{s}")
                  for s in range(NLOC)}
            ods = {s: pk.tile([P, SROW], BF16, tag=f"od{s}", name=f"od{s}")
                   for s in range(NLOC)}

            ya0 = pps.tile([P, HB], F32, tag="ya0")
            yb0 = pps.tile([P, HB], F32, tag="yb0")
            y1 = pps.tile([P, COLS], F32, tag="y1")
            y2 = pps.tile([P, COLS], F32, tag="y2")
            ya3 = pps.tile([P, HB], F32, tag="ya3")
            yb3 = pps.tile([P, HB], F32, tag="yb3")

            def wgt(s):
                if s == 0:
                    return wm0[:, 0:P], wm0[:, P : 2 * P]
                o = (s - 1) * 2 * P
                return wmr[:, o : o + P], wmr[:, o + P : o + 2 * P]

            def mm_bank(yt, w0, w1, srcs, bank):
                # yt: [P, HB] psum slice; srcs: list of (tile, col_offset)
                o0 = bank * HB
                lo = 1 if bank == 0 else 0
                n = len(srcs)
                for i, (sq, co) in enumerate(srcs):
                    o = co + o0
                    nc.tensor.matmul(yt[:, 0:HB], w0, sq[:, o : o + HB],
                                     start=i == 0, stop=False)
                    nc.tensor.matmul(yt[:, lo:HB], w1,
                                     sq[:, o + lo - 1 : o + HB - 1],
                                     start=False, stop=i == n - 1)

            def emit_mm(s):
                w0, w1 = wgt(s)
                if s == 0:
                    srcs = [(sq0[0], 0), (sq0[1], 0)]
                    mm_bank(ya0, w0, w1, srcs, 0)
                    mm_bank(yb0, w0, w1, srcs, 1)
                elif s == 3:
                    srcs = [(sqs[s], 0), (sqs[s], COLS)]
                    mm_bank(ya3, w0, w1, srcs, 0)
                    mm_bank(yb3, w0, w1, srcs, 1)
                else:
                    y = y1 if s == 1 else y2
                    srcs = [(sqs[s], 0), (sqs[s], COLS)]
                    mm_bank(y[:, 0:HB], w0, w1, srcs, 0)
                    mm_bank(y[:, HB:COLS], w0, w1, srcs, 1)

            def emit_ln(s, half=None):
                # ap = Ln(scale*env + bias) = LG - T + W  (b1 folded on host)
                ap = aps[s]
                if half is None:
                    y = y1 if s == 1 else y2
                    nc.scalar.activation(ap[:], y[:], AF.Ln,
                                         bias=col(s, PRM_LNBIAS),
                                         scale=col(s, PRM_LNSCALE))
                else:
                    yt = {(0, 0): ya0, (0, 1): yb0,
                          (3, 0): ya3, (3, 1): yb3}[(s, half)]
                    nc.scalar.activation(ap[:, half * HB : half * HB + HB],
                                         yt[:], AF.Ln,
                                         bias=col(s, PRM_LNBIAS),
                                         scale=col(s, PRM_LNSCALE))

            def emit_knee(s, c0=0, c1=COLS):
                nc.vector._custom_dve(
                    KNEE_OP, out=Qs[s][:, c0:c1], in0=aps[s][:, c0:c1],
                    s0=col(s, PRM_W2),
                )

            def emit_exp(s, c0=0, c1=COLS):
                nc.scalar.activation(gs[s][:, c0:c1], Qs[s][:, c0:c1], AF.Exp,
                                     scale=col(s, PRM_C4W))

            def emit_out(s, c0=0, c1=COLS):
                od = ods[s]
                w = c1 - c0
                o0 = od[:, c0 : c0 + w]
                o3 = bass.AP(o0.tensor, o0.offset,
                             [list(o0.ap[0]), [COLS, C], [1, w]])
                x0 = xt[s][:, c0 : c0 + w]
                x3 = bass.AP(x0.tensor, x0.offset,
                             [list(x0.ap[0]), [COLS, C], [1, w]])
                g0 = gs[s][:, c0 : c0 + w]
                g3 = bass.AP(g0.tensor, g0.offset,
                             [list(g0.ap[0]), [0, C], [1, w]])
                nc.vector.tensor_tensor(o3, g3, x3, OP.mult)

            def emit_store(s, eng, rows=slice(0, P)):
                dsl = slice(s * SROW, (s + 1) * SROW)
                eng.dma_start(out=od_ext[rows, dsl], in_=ods[s][rows, :])

            def emit_out3(h):
                # s3 out, half-major od layout: half h = [ch0 512 | ch1 512]
                # at cols [h*1024, (h+1)*1024) -- keeps its store a plain 2D
                # contiguous DMA (2KB rows) instead of a 1KB-chunk 3D one.
                od = ods[3]
                o0 = od[:, h * COLS : h * COLS + HB]
                o3 = bass.AP(o0.tensor, o0.offset,
                             [list(o0.ap[0]), [HB, C], [1, HB]])
                x0 = xt[3][:, h * HB : h * HB + HB]
                x3 = bass.AP(x0.tensor, x0.offset,
                             [list(x0.ap[0]), [COLS, C], [1, HB]])
                g0 = gs[3][:, h * HB : h * HB + HB]
                g3 = bass.AP(g0.tensor, g0.offset,
                             [list(g0.ap[0]), [0, C], [1, HB]])
                nc.vector.tensor_tensor(o3, g3, x3, OP.mult)

            def emit_store3(h, eng, rows=slice(0, P)):
                dsl = slice(3 * SROW + h * COLS, 3 * SROW + (h + 1) * COLS)
                eng.dma_start(out=od_ext[rows, dsl],
                              in_=ods[3][rows, h * COLS : (h + 1) * COLS])

            def emit_store_half(s, h, eng):
                od = ods[s]
                o0 = od[:, h * HB : h * HB + HB]
                src3 = bass.AP(o0.tensor, o0.offset,
                               [list(o0.ap[0]), [COLS, C], [1, HB]])
                d0 = od_ext[:, s * SROW + h * HB : s * SROW + h * HB + HB]
                dst3 = bass.AP(d0.tensor, d0.offset,
                               [list(d0.ap[0]), [COLS, C], [1, HB]])
                eng.dma_start(out=dst3, in_=src3)

            def emit_sq(s):
                nc.vector.tensor_tensor(sqs[s][:], xt[s][:], xt[s][:], OP.mult)

            # ---- pipelined emission (steps pin per-engine queue order) ---
            with step(1):
                nc.vector.tensor_tensor(sq0[0][:], xt[0][:, 0:COLS],
                                        xt[0][:, 0:COLS], OP.mult)
            with step(2):
                nc.vector.tensor_tensor(sq0[1][:], xt[0][:, COLS:SROW],
                                        xt[0][:, COLS:SROW], OP.mult)
            with step(3):
                emit_mm(0)
                emit_sq(1)
            with step(4):
                emit_ln(0, half=0)
                emit_mm(1)
            with step(4.1):
                emit_ln(0, half=1)
            with step(5):
                emit_knee(0)
            with step(5.2):
                emit_sq(2)
            with step(5.5):
                emit_exp(0)
            with step(6):
                emit_ln(1)
                emit_mm(2)
            with step(6.5):
                emit_sq(3)
            with step(7):
                emit_knee(1)
                emit_mm(3)
            with step(7.5):
                emit_out(0)
                emit_ln(2)
            H0, H1 = slice(0, 64), slice(64, P)
            with step(8):
                emit_exp(1)
                emit_store(0, nc.sync, H0)
            with step(8.05):
                emit_store(0, nc.gpsimd, H1)
            with step(8.5):
                emit_ln(3, half=0)
            with step(8.6):
                emit_ln(3, half=1)
            with step(9):
                emit_knee(2)
            with step(9.5):
                emit_out(1)
                emit_exp(2)
            with step(9.7):
                emit_store(1, nc.sync, H0)
            with step(9.75):
                emit_store(1, nc.gpsimd, H1)
            with step(10):
                emit_knee(3, 0, HB)
            with step(10.2):
                emit_knee(3, HB, COLS)
            with step(10.5):
                emit_exp(3, 0, HB)
            with step(10.7):
                emit_exp(3, HB, COLS)
            with step(11):
                emit_out(2)
            with step(11.2):
                emit_store(2, nc.sync, H0)
            with step(11.25):
                emit_store(2, nc.gpsimd, H1)
            with step(11.5):
                emit_out3(0)
            with step(11.7):
                emit_store3(0, nc.scalar, H0)
            with step(11.75):
                emit_store3(0, nc.sync, H1)
            with step(12):
                emit_out3(1)
            with step(12.05):
                emit_store3(1, nc.sync, H0)
            with step(12.1):
                emit_store3(1, nc.scalar, H1)

    nc.finalize()
    return nc


def host_params(z_alpha, log_threshold, log_ratio, log_knee):
    z = z_alpha.astype(np.float64).reshape(-1)
    alpha = 1.0 / (1.0 + np.exp(-z))
    aK = np.exp(K_FIR * np.log(alpha))
    assert np.all(aK < 1e-6), "FIR tail non-negligible; needs shift correction"
    aJ = np.exp(JF * np.log(alpha))
    assert np.all(aJ < 1e-7), "block-conv truncation at 256 taps too short"
    T = log_threshold.astype(np.float64).reshape(-1) - 6.0
    R = 1.0 + np.exp(log_ratio.astype(np.float64).reshape(-1))
    W = np.exp(log_knee.astype(np.float64).reshape(-1))
    c = 1.0 / R - 1.0

    n = alpha.shape[0]
    prms, wms = [], []
    dp = np.arange(P)[:, None] - np.arange(P)[None, :]
    pq = -dp  # pq[q, p] = p - q
    for c0 in range(n // NLOC):
        sl = slice(c0 * NLOC, (c0 + 1) * NLOC)
        a4, T4, W4, c4 = alpha[sl], T[sl], W[sl], c[sl]
        prm = np.zeros((P, NLOC * NPRM), np.float64)
        wm = np.zeros((P, NLOC * 2 * P), np.float64)
        for s in range(NLOC):
            o = s * NPRM
            eb1 = np.exp(W4[s] - T4[s])  # fold b1 = W - T into Ln
            prm[:, o + PRM_LNSCALE] = 0.5 * (1.0 - a4[s]) * eb1
            prm[:, o + PRM_LNBIAS] = EPS * eb1
            prm[:, o + PRM_W2] = 2.0 * W4[s]
            prm[:, o + PRM_C4W] = c4[s] / (4.0 * W4[s])
            la = np.log(a4[s])
            e0 = pq * la
            w0 = np.where((pq >= 0) & (e0 > -100.0), np.exp(e0), 0.0)
            e1 = (P + pq) * la
            w1 = np.where(e1 > -100.0, np.exp(e1), 0.0)
            wm[:, s * 2 * P : s * 2 * P + P] = w0
            wm[:, s * 2 * P + P : s * 2 * P + 2 * P] = w1
        prms.append(prm.astype(np.float32))
        wms.append(wm.astype(np.float32).astype(ml_dtypes.bfloat16))
    return prms, wms


def shuffle_in(x_core):
    """(NLOC, C, L) f32 -> (P, ROW) bf16 device layout (time-on-partition)."""
    xb = x_core.astype(np.float32).astype(ml_dtypes.bfloat16)
    v = xb.reshape(NLOC, C, COLS, P).transpose(3, 0, 1, 2)
    return np.ascontiguousarray(v.reshape(P, ROW))


def unshuffle_out(od):
    """(P, ROW) bf16 device layout -> (NLOC, C, L) f32.

    Samples 0-2 are channel-major ([ch0 1024 | ch1 1024]); sample 3 is
    half-major ([h0: ch0 512, ch1 512 | h1: ch0 512, ch1 512]) so its
    stores are contiguous 2D DMAs."""
    v = od.reshape(P, NLOC, C, COLS).astype(np.float32)
    out = v.transpose(1, 2, 3, 0).reshape(NLOC, C, L)
    v3 = od.reshape(P, NLOC, 2 * C, COLS // 2)[:, 3].astype(np.float32)
    v3 = v3.reshape(P, 2, C, COLS // 2)  # [p, half, ch, j]
    out[3] = v3.transpose(2, 1, 3, 0).reshape(C, L)
    return out


def _ensure_ntff_hook():
    import types

    try:
        from antenv.axon_hooks import get_axon_ntff_profile_hook  # noqa: F401

        return
    except ImportError:
        pass
    try:
        from trn_agent_boot.trn_boot import _ntff_profile_via_ctypes
    except ImportError:
        return
    hook = _ntff_profile_via_ctypes("/opt/axon/libaxon_pjrt.so")
    mod = types.ModuleType("antenv.axon_hooks")
    mod._hook = hook
    mod.get_axon_ntff_profile_hook = lambda: mod._hook

    def set_axon_ntff_profile_hook(h):
        mod._hook = h

    mod.set_axon_ntff_profile_hook = set_axon_ntff_profile_hook
    import antenv

    sys.modules["antenv.axon_hooks"] = mod
    antenv.axon_hooks = mod


def kernel(input_signals, z_alpha, log_threshold, log_ratio, log_knee):
    from concourse.bass_utils import run_bass_kernel_spmd

    x = np.asarray(input_signals, np.float32)
    prms, wms = host_params(
        np.asarray(z_alpha), np.asarray(log_threshold),
        np.asarray(log_ratio), np.asarray(log_knee),
    )

    nc = build_nc()
    core_ids = list(range(NCORES))
    in_maps = [
        {
            "xd": shuffle_in(x[i * NLOC : (i + 1) * NLOC]),
            "prm": prms[i],
            "wm": wms[i],
        }
        for i in core_ids
    ]

    trace = os.environ.get("BASS_KERNEL_TRACE", "0") == "1"
    if trace:
        _ensure_ntff_hook()
    res = run_bass_kernel_spmd(nc, in_maps, core_ids, trace=trace)
    if trace:
        TRACE_RESULT["exec_time_ns"] = res.exec_time_ns
        TRACE_RESULT["results"] = res

    out = np.empty((N, C, L), np.float32)
    for i in core_ids:
        out[i * NLOC : (i + 1) * NLOC] = unshuffle_out(
            np.asarray(res.results[i]["od"])
        )
    return out


# revision 18
# speedup vs baseline: 1.0219x; 1.0219x over previous
"""Trainium2 Bass kernel for nn_ApproxCompressor (v14).

Reference (per sample n):
    alpha = sigmoid(z_alpha); h[k] = (1-alpha)*alpha^k (k<16384)
    env   = causal_conv(mean_c x^2, h); LG = log(env + 1e-5)
    quadratic-knee gain; out = gain * x.

Kernel strategy (8 cores x 4 samples, pure data parallel):
  * Time-on-partition layout: sample time t = 128*j + p (partition p,
    column j; 1024 columns per sample).  The exponential FIR truncated at
    256 taps (a^256 < 1e-7, asserted on host) is EXACTLY a 2-tap block
    convolution over columns:
        Y[:, j] = W0 @ D[:, j] + W1 @ D[:, j-1]
    with W0[q,p] = a^(p-q) (p>=q), W1[q,p] = a^(128+p-q), run as bf16
    matmuls on the PE accumulating in PSUM (per-channel squares matmul'd
    separately so no channel pre-add is needed).
  * Relu-free knee: host folds b1 = W - T into the Ln activation
    (scale' = 0.5*(1-alpha)*e^b1, bias' = eps*e^b1) so ACT's Ln emits
    A' = LG - T + W directly from PSUM.  A SINGLE custom DVE op
    (registered at import into dve_ops.OPS) then computes
        Q = c*(2A' - c),  c = min(relu(A'), 2W)
    which equals relu(u+W)^2 - relu(u-W)^2 exactly (c=0 kills the A'<0
    region), entirely in the DVE's fp32 pipeline.  log_gain = c/4W * Q
    is folded into Exp's input scale.  Per sample the DVE runs only
    square (TT 2x), knee (1 custom op), out-mult (TT 2x) ~= 3.7us;
    ACT runs Ln + Exp ~= 2.4us.
  * The Tile scheduler is steered with tile_wait_until step numbers
    (pure sim-time priorities) -- v6 showed it otherwise reorders the
    per-engine queues around its DMA model and stalls the pipeline.
  * DMA (3 rings, ~770KB each input side): sync x0c0,x1c0,x2c0;
    scalar wm0,prm,wmr,x3c0,x3c1 (weights lead so the first matmul is
    never weight-blocked); pool x0c1,x1c1,x2c1.  Stores issue per
    sample as outputs complete (sync/pool), s3's final half on scalar.
  * s0 and s3 are processed in 512-column (PSUM-bank) halves through
    Ln/knee (s3 also Exp/out) to shorten pipeline fill and drain.
"""

import os
import sys

import numpy as np


def _import_concourse():
    try:
        import concourse.bass  # noqa: F401
    except ImportError:
        for p in ("/opt/trn_rl_repo", "/root/.axon_site/_ro/trn_rl_repo"):
            if os.path.isdir(p) and p not in sys.path:
                sys.path.insert(0, p)
        import concourse.bass  # noqa: F401


_import_concourse()

import ml_dtypes  # noqa: E402
import concourse.bass as bass  # noqa: E402
import concourse.tile as tile  # noqa: E402
from concourse import bacc, mybir  # noqa: E402
from concourse import dve_ops as _dvo  # noqa: E402
from concourse.dve_spec import (  # noqa: E402
    C0 as _C0,
    Spec as _Spec,
    Src0 as _Src0,
    lower as _dve_lower,
    minn as _minn,
    relu as _relu,
)
from concourse.dve_uop import DveOpSpec as _DveOpSpec  # noqa: E402

N, C, L = 32, 2, 131072
NCORES = 8
NLOC = N // NCORES  # 4 samples/core
P = 128
COLS = L // P  # 1024 columns per sample
SROW = C * COLS  # 2048 elems per sample per partition row
ROW = NLOC * SROW  # 8192 elems per partition row
EPS = 1e-5
K_FIR = 16384
JF = 256  # FIR truncation (2 blocks of 128)
HB = COLS // 2  # psum bank = 512 f32 columns

F32 = mybir.dt.float32
BF16 = mybir.dt.bfloat16

# per-sample param column slots (prm tile is [P, NLOC*NPRM] f32)
PRM_LNSCALE, PRM_LNBIAS, PRM_W2, PRM_C4W = 0, 1, 2, 3
NPRM = 4
ACT_SET_ID = 6  # natural_log_exp_and_others: ln, exp

TRACE_RESULT = {}


def _knee_ref(in0, in1, s0, s1, imm2):
    x = in0.astype(np.float32)
    c = np.clip(x, 0.0, s0)
    return c * (2.0 * x - c)


def _register_knee_op():
    """Append the fused knee op  Q = c*(2A'-c), c = clip(A', 0, s0)  to
    dve_ops.OPS (the documented extension point for custom DVE ucode).
    Idempotent; shas computed from lower() so they always match."""
    name = "ACOMP_KNEE_V1"
    for op in _dvo.OPS:
        if op.name == name:
            return op
    c = _minn(_relu(_Src0), _C0)
    spec = _Spec(body=c * (_Src0 + _Src0 - c), reference=_knee_ref)
    row = _dvo._CUSTOM_DVE_ROW_BASE + len(_dvo.OPS)
    assert row < 0x20, "custom DVE opcode rows exhausted"
    _dvo._SUB_OPCODE_FOR_NAME[name] = row
    shas = {}
    for ver in ("v3", "v4"):
        s = _DveOpSpec(name=name, opcode=row,
                       uops=_dve_lower(spec, ver=ver), rd1_en=False)
        shas[ver] = s.sha(ver)
    op = _dvo.DveOp(name, spec, subdim=False, uops_sha=shas)
    _dvo.OPS.append(op)
    _dvo.CUSTOM_DVE_SPECS[name] = spec
    return op


KNEE_OP = _register_knee_op()


def build_nc():
    AF = mybir.ActivationFunctionType
    OP = mybir.AluOpType

    nc = bacc.Bacc("TRN2", target_bir_lowering=False, num_devices=NCORES)
    xd_ext = nc.declare_dram_parameter("xd", [P, ROW], BF16, isOutput=False)
    prm_ext = nc.declare_dram_parameter("prm", [P, NLOC * NPRM], F32, isOutput=False)
    wm_ext = nc.declare_dram_parameter("wm", [P, NLOC * 2 * P], BF16, isOutput=False)
    od_ext = nc.declare_dram_parameter("od", [P, ROW], BF16, isOutput=True)

    with tile.TileContext(nc) as tc:
        atl = mybir.InstLoadActFuncSet(
            name=nc.get_next_instruction_name(), ins=[], outs=[],
            act_func_set_id=ACT_SET_ID,
        )
        nc.scalar.add_instruction(atl)
        step = tc.tile_wait_until
        with (
            tc.tile_pool(name="pc", bufs=1) as pc,
            tc.tile_pool(name="pin", bufs=1) as pin,
            tc.tile_pool(name="pk", bufs=1) as pk,
            tc.tile_pool(name="pps", bufs=1, space=bass.MemorySpace.PSUM) as pps,
        ):
            wm0 = pc.tile([P, 2 * P], BF16, tag="wm0")
            wmr = pc.tile([P, 3 * 2 * P], BF16, tag="wmr")
            prm = pc.tile([P, NLOC * NPRM], F32, tag="prm")

            def col(s, k):
                return prm[:, s * NPRM + k : s * NPRM + k + 1]

            xt = [pin.tile([P, SROW], BF16, tag=f"x{s}", name=f"x{s}")
                  for s in range(NLOC)]

            def ldch(s, ch, eng):
                c0 = s * SROW + ch * COLS
                eng.dma_start(out=xt[s][:, ch * COLS : (ch + 1) * COLS],
                              in_=xd_ext[:, c0 : c0 + COLS])

            # ---- input DMA (ring order fixed by step numbers) ------------
            with step(0.01):
                ldch(0, 0, nc.sync)
                nc.scalar.dma_start(out=wm0[:], in_=wm_ext[:, 0 : 2 * P])
                ldch(0, 1, nc.gpsimd)
            with step(0.02):
                nc.scalar.dma_start(out=prm[:], in_=prm_ext[:])
            with step(0.03):
                ldch(1, 0, nc.sync)
                ldch(1, 1, nc.gpsimd)
                nc.scalar.dma_start(out=wmr[:], in_=wm_ext[:, 2 * P :])
            with step(0.05):
                ldch(2, 0, nc.sync)
                ldch(2, 1, nc.gpsimd)
            with step(0.06):
                ldch(3, 0, nc.scalar)
            with step(0.07):
                ldch(3, 1, nc.scalar)

            # ---- tiles ---------------------------------------------------
            sq0 = [pk.tile([P, COLS], BF16, tag=f"sq0c{c_}", name=f"sq0c{c_}")
                   for c_ in range(2)]
            sqs = {s: pk.tile([P, SROW], BF16, tag=f"sq{s}", name=f"sq{s}")
                   for s in range(1, NLOC)}
            aps = {s: pk.tile([P, COLS], BF16, tag=f"ap{s}", name=f"ap{s}")
                   for s in range(NLOC)}
            Qs = {s: pk.tile([P, COLS], BF16, tag=f"Q{s}", name=f"Q{s}")
                  for s in range(NLOC)}
            gs = {s: pk.tile([P, COLS], BF16, tag=f"g{s}", name=f"g{s}")
                  for s in range(NLOC)}
            ods = {s: pk.tile([P, SROW], BF16, tag=f"od{s}", name=f"od{s}")
                   for s in range(NLOC)}

            ya0 = pps.tile([P, HB], F32, tag="ya0")
            yb0 = pps.tile([P, HB], F32, tag="yb0")
            y1 = pps.tile([P, COLS], F32, tag="y1")
            y2 = pps.tile([P, COLS], F32, tag="y2")
            ya3 = pps.tile([P, HB], F32, tag="ya3")
            yb3 = pps.tile([P, HB], F32, tag="yb3")

            def wgt(s):
                if s == 0:
                    return wm0[:, 0:P], wm0[:, P : 2 * P]
                o = (s - 1) * 2 * P
                return wmr[:, o : o + P], wmr[:, o + P : o + 2 * P]

            def mm_bank(yt, w0, w1, srcs, bank):
                # yt: [P, HB] psum slice; srcs: list of (tile, col_offset)
                o0 = bank * HB
                lo = 1 if bank == 0 else 0
                n = len(srcs)
                for i, (sq, co) in enumerate(srcs):
                    o = co + o0
                    nc.tensor.matmul(yt[:, 0:HB], w0, sq[:, o : o + HB],
                                     start=i == 0, stop=False)
                    nc.tensor.matmul(yt[:, lo:HB], w1,
                                     sq[:, o + lo - 1 : o + HB - 1],
                                     start=False, stop=i == n - 1)

            def emit_mm(s):
                w0, w1 = wgt(s)
                if s == 0:
                    srcs = [(sq0[0], 0), (sq0[1], 0)]
                    mm_bank(ya0, w0, w1, srcs, 0)
                    mm_bank(yb0, w0, w1, srcs, 1)
                elif s == 3:
                    srcs = [(sqs[s], 0), (sqs[s], COLS)]
                    mm_bank(ya3, w0, w1, srcs, 0)
                    mm_bank(yb3, w0, w1, srcs, 1)
                else:
                    y = y1 if s == 1 else y2
                    srcs = [(sqs[s], 0), (sqs[s], COLS)]
                    mm_bank(y[:, 0:HB], w0, w1, srcs, 0)
                    mm_bank(y[:, HB:COLS], w0, w1, srcs, 1)

            def emit_ln(s, half=None):
                # ap = Ln(scale*env + bias) = LG - T + W  (b1 folded on host)
                ap = aps[s]
                if half is None:
                    y = y1 if s == 1 else y2
                    nc.scalar.activation(ap[:], y[:], AF.Ln,
                                         bias=col(s, PRM_LNBIAS),
                                         scale=col(s, PRM_LNSCALE))
                else:
                    yt = {(0, 0): ya0, (0, 1): yb0,
                          (3, 0): ya3, (3, 1): yb3}[(s, half)]
                    nc.scalar.activation(ap[:, half * HB : half * HB + HB],
                                         yt[:], AF.Ln,
                                         bias=col(s, PRM_LNBIAS),
                                         scale=col(s, PRM_LNSCALE))

            def emit_knee(s, c0=0, c1=COLS):
                nc.vector._custom_dve(
                    KNEE_OP, out=Qs[s][:, c0:c1], in0=aps[s][:, c0:c1],
                    s0=col(s, PRM_W2),
                )

            def emit_exp(s, c0=0, c1=COLS):
                nc.scalar.activation(gs[s][:, c0:c1], Qs[s][:, c0:c1], AF.Exp,
                                     scale=col(s, PRM_C4W))

            def emit_out(s, c0=0, c1=COLS):
                od = ods[s]
                w = c1 - c0
                o0 = od[:, c0 : c0 + w]
                o3 = bass.AP(o0.tensor, o0.offset,
                             [list(o0.ap[0]), [COLS, C], [1, w]])
                x0 = xt[s][:, c0 : c0 + w]
                x3 = bass.AP(x0.tensor, x0.offset,
                             [list(x0.ap[0]), [COLS, C], [1, w]])
                g0 = gs[s][:, c0 : c0 + w]
                g3 = bass.AP(g0.tensor, g0.offset,
                             [list(g0.ap[0]), [0, C], [1, w]])
                nc.vector.tensor_tensor(o3, g3, x3, OP.mult)

            def emit_store(s, eng, rows=slice(0, P)):
                dsl = slice(s * SROW, (s + 1) * SROW)
                eng.dma_start(out=od_ext[rows, dsl], in_=ods[s][rows, :])

            def emit_out3(h):
                # s3 out, half-major od layout: half h = [ch0 512 | ch1 512]
                # at cols [h*1024, (h+1)*1024) -- keeps its store a plain 2D
                # contiguous DMA (2KB rows) instead of a 1KB-chunk 3D one.
                od = ods[3]
                o0 = od[:, h * COLS : h * COLS + HB]
                o3 = bass.AP(o0.tensor, o0.offset,
                             [list(o0.ap[0]), [HB, C], [1, HB]])
                x0 = xt[3][:, h * HB : h * HB + HB]
                x3 = bass.AP(x0.tensor, x0.offset,
                             [list(x0.ap[0]), [COLS, C], [1, HB]])
                g0 = gs[3][:, h * HB : h * HB + HB]
                g3 = bass.AP(g0.tensor, g0.offset,
                             [list(g0.ap[0]), [0, C], [1, HB]])
                nc.vector.tensor_tensor(o3, g3, x3, OP.mult)

            def emit_store3(h, eng):
                dsl = slice(3 * SROW + h * COLS, 3 * SROW + (h + 1) * COLS)
                eng.dma_start(out=od_ext[:, dsl],
                              in_=ods[3][:, h * COLS : (h + 1) * COLS])

            def emit_store_half(s, h, eng):
                od = ods[s]
                o0 = od[:, h * HB : h * HB + HB]
                src3 = bass.AP(o0.tensor, o0.offset,
                               [list(o0.ap[0]), [COLS, C], [1, HB]])
                d0 = od_ext[:, s * SROW + h * HB : s * SROW + h * HB + HB]
                dst3 = bass.AP(d0.tensor, d0.offset,
                               [list(d0.ap[0]), [COLS, C], [1, HB]])
                eng.dma_start(out=dst3, in_=src3)

            def emit_sq(s):
                nc.vector.tensor_tensor(sqs[s][:], xt[s][:], xt[s][:], OP.mult)

            # ---- pipelined emission (steps pin per-engine queue order) ---
            with step(1):
                nc.vector.tensor_tensor(sq0[0][:], xt[0][:, 0:COLS],
                                        xt[0][:, 0:COLS], OP.mult)
            with step(2):
                nc.vector.tensor_tensor(sq0[1][:], xt[0][:, COLS:SROW],
                                        xt[0][:, COLS:SROW], OP.mult)
            with step(3):
                emit_mm(0)
                emit_sq(1)
            with step(4):
                emit_ln(0, half=0)
                emit_mm(1)
            with step(4.1):
                emit_ln(0, half=1)
            with step(5):
                emit_knee(0, 0, HB)
            with step(5.1):
                emit_knee(0, HB, COLS)
            with step(5.2):
                emit_sq(2)
            with step(5.5):
                emit_exp(0)
            with step(6):
                emit_ln(1)
                emit_mm(2)
            with step(6.5):
                emit_sq(3)
            with step(7):
                emit_knee(1)
                emit_mm(3)
            with step(7.5):
                emit_out(0)
                emit_ln(2)
            H0, H1 = slice(0, 64), slice(64, P)
            with step(8):
                emit_exp(1)
                emit_store(0, nc.sync, H0)
            with step(8.05):
                emit_store(0, nc.gpsimd, H1)
            with step(8.5):
                emit_ln(3, half=0)
            with step(8.6):
                emit_ln(3, half=1)
            with step(9):
                emit_knee(2)
            with step(9.5):
                emit_out(1)
                emit_exp(2)
            with step(9.7):
                emit_store(1, nc.sync, H0)
            with step(9.75):
                emit_store(1, nc.gpsimd, H1)
            with step(10):
                emit_knee(3, 0, HB)
            with step(10.2):
                emit_knee(3, HB, COLS)
            with step(10.5):
                emit_exp(3, 0, HB)
            with step(10.7):
                emit_exp(3, HB, COLS)
            with step(11):
                emit_out(2)
            with step(11.2):
                emit_store(2, nc.sync, H0)
            with step(11.25):
                emit_store(2, nc.gpsimd, H1)
            with step(11.5):
                emit_out3(0)
            with step(11.7):
                emit_store3(0, nc.scalar)
            with step(12):
                emit_out3(1)
            with step(12.05):
                emit_store3(1, nc.sync)

    nc.finalize()
    return nc


def host_params(z_alpha, log_threshold, log_ratio, log_knee):
    z = z_alpha.astype(np.float64).reshape(-1)
    alpha = 1.0 / (1.0 + np.exp(-z))
    aK = np.exp(K_FIR * np.log(alpha))
    assert np.all(aK < 1e-6), "FIR tail non-negligible; needs shift correction"
    aJ = np.exp(JF * np.log(alpha))
    assert np.all(aJ < 1e-7), "block-conv truncation at 256 taps too short"
    T = log_threshold.astype(np.float64).reshape(-1) - 6.0
    R = 1.0 + np.exp(log_ratio.astype(np.float64).reshape(-1))
    W = np.exp(log_knee.astype(np.float64).reshape(-1))
    c = 1.0 / R - 1.0

    n = alpha.shape[0]
    prms, wms = [], []
    dp = np.arange(P)[:, None] - np.arange(P)[None, :]
    pq = -dp  # pq[q, p] = p - q
    for c0 in range(n // NLOC):
        sl = slice(c0 * NLOC, (c0 + 1) * NLOC)
        a4, T4, W4, c4 = alpha[sl], T[sl], W[sl], c[sl]
        prm = np.zeros((P, NLOC * NPRM), np.float64)
        wm = np.zeros((P, NLOC * 2 * P), np.float64)
        for s in range(NLOC):
            o = s * NPRM
            eb1 = np.exp(W4[s] - T4[s])  # fold b1 = W - T into Ln
            prm[:, o + PRM_LNSCALE] = 0.5 * (1.0 - a4[s]) * eb1
            prm[:, o + PRM_LNBIAS] = EPS * eb1
            prm[:, o + PRM_W2] = 2.0 * W4[s]
            prm[:, o + PRM_C4W] = c4[s] / (4.0 * W4[s])
            la = np.log(a4[s])
            e0 = pq * la
            w0 = np.where((pq >= 0) & (e0 > -100.0), np.exp(e0), 0.0)
            e1 = (P + pq) * la
            w1 = np.where(e1 > -100.0, np.exp(e1), 0.0)
            wm[:, s * 2 * P : s * 2 * P + P] = w0
            wm[:, s * 2 * P + P : s * 2 * P + 2 * P] = w1
        prms.append(prm.astype(np.float32))
        wms.append(wm.astype(np.float32).astype(ml_dtypes.bfloat16))
    return prms, wms


def shuffle_in(x_core):
    """(NLOC, C, L) f32 -> (P, ROW) bf16 device layout (time-on-partition)."""
    xb = x_core.astype(np.float32).astype(ml_dtypes.bfloat16)
    v = xb.reshape(NLOC, C, COLS, P).transpose(3, 0, 1, 2)
    return np.ascontiguousarray(v.reshape(P, ROW))


def unshuffle_out(od):
    """(P, ROW) bf16 device layout -> (NLOC, C, L) f32.

    Samples 0-2 are channel-major ([ch0 1024 | ch1 1024]); sample 3 is
    half-major ([h0: ch0 512, ch1 512 | h1: ch0 512, ch1 512]) so its
    stores are contiguous 2D DMAs."""
    v = od.reshape(P, NLOC, C, COLS).astype(np.float32)
    out = v.transpose(1, 2, 3, 0).reshape(NLOC, C, L)
    v3 = od.reshape(P, NLOC, 2 * C, COLS // 2)[:, 3].astype(np.float32)
    v3 = v3.reshape(P, 2, C, COLS // 2)  # [p, half, ch, j]
    out[3] = v3.transpose(2, 1, 3, 0).reshape(C, L)
    return out


def _ensure_ntff_hook():
    import types

    try:
        from antenv.axon_hooks import get_axon_ntff_profile_hook  # noqa: F401

        return
    except ImportError:
        pass
    try:
        from trn_agent_boot.trn_boot import _ntff_profile_via_ctypes
    except ImportError:
        return
    hook = _ntff_profile_via_ctypes("/opt/axon/libaxon_pjrt.so")
    mod = types.ModuleType("antenv.axon_hooks")
    mod._hook = hook
    mod.get_axon_ntff_profile_hook = lambda: mod._hook

    def set_axon_ntff_profile_hook(h):
        mod._hook = h

    mod.set_axon_ntff_profile_hook = set_axon_ntff_profile_hook
    import antenv

    sys.modules["antenv.axon_hooks"] = mod
    antenv.axon_hooks = mod


def kernel(input_signals, z_alpha, log_threshold, log_ratio, log_knee):
    from concourse.bass_utils import run_bass_kernel_spmd

    x = np.asarray(input_signals, np.float32)
    prms, wms = host_params(
        np.asarray(z_alpha), np.asarray(log_threshold),
        np.asarray(log_ratio), np.asarray(log_knee),
    )

    nc = build_nc()
    core_ids = list(range(NCORES))
    in_maps = [
        {
            "xd": shuffle_in(x[i * NLOC : (i + 1) * NLOC]),
            "prm": prms[i],
            "wm": wms[i],
        }
        for i in core_ids
    ]

    trace = os.environ.get("BASS_KERNEL_TRACE", "0") == "1"
    if trace:
        _ensure_ntff_hook()
    res = run_bass_kernel_spmd(nc, in_maps, core_ids, trace=trace)
    if trace:
        TRACE_RESULT["exec_time_ns"] = res.exec_time_ns
        TRACE_RESULT["results"] = res

    out = np.empty((N, C, L), np.float32)
    for i in core_ids:
        out[i * NLOC : (i + 1) * NLOC] = unshuffle_out(
            np.asarray(res.results[i]["od"])
        )
    return out
